# revision 1
# baseline (speedup 1.0000x reference)
"""Trainium2 Bass kernel for nn_DemLocGraphEncoder (4-layer GIN + variational heads).

Strategy
--------
The GIN segment-sum aggregation is recast as a dense matmul with a
host-precomputed (I + A)^T adjacency-multiplicity matrix (N=8192, so the
dense form maps perfectly onto the 128x128 TensorEngine; avg degree 32
makes gather/scatter paths no faster and far more complex).

Sharding: nodes are row-sharded 1024/core across 8 cores.  Each layer:
  1. AllGather node features x (node-major) -> x_full  [skipped for layer 0,
     whose input is replicated to every core]
  2. agg^T = x_full^T @ ATshard  on TensorE (feature-major output)
  3. MLP entirely in feature-major form: hT = relu(W1-matmuls + b1),
     xT = relu(W2-matmuls + b2)  (weights replicated, used directly as lhsT)
  4. PE-transpose xT -> node-major x_own, DMA to DRAM for the next AllGather.
Layer 3 folds w2_3 @ {wm,wv} into two fused [2048,128] heads (x3 is never
materialized), then z = mean + var*eps on the VectorEngine.

All matmuls run in fp16 (1 cycle/row on TRN2 PE, fp32 PSUM accumulation;
fp16 chosen over bf16 for its 10-bit mantissa; activations stay < ~1e3 so
no overflow).  Outputs are fp32.
"""

import sys

if "/opt/trn_rl_repo" not in sys.path:
    sys.path.insert(0, "/opt/trn_rl_repo")

import numpy as np

N, E, T, H, O, L = 8192, 262144, 256, 2048, 1024, 128
NC = 8
NS = N // NC          # 1024 nodes per core
P = 128
KT_NODES = N // P     # 64 k-tiles over source nodes
ND = NS // 512        # 2 free-dim tiles over own nodes

_PROGRAM_CACHE = {}


def _build_program(collectives=True, opts=None):
    opts = dict(opts or {})
    no_transpose = opts.get("no_transpose", False)   # sim-only: DMA instead of PE transpose
    drain_split = opts.get("drain_split", True)     # alternate agg drains DVE/ACT
    at_bufs = opts.get("at_bufs", 14)
    x_bufs = opts.get("x_bufs", 7)
    w_bufs = opts.get("w_bufs", 6)
    ps_bufs = opts.get("ps_bufs", 8)
    agg_group = opts.get("agg_group", 8)
    dma_tp = opts.get("dma_tp", False)     # feature-major AG + XBAR-transposed x loads
    l0_split = opts.get("l0_split", True)  # pipeline layer-0 n-halves with MLP
    import concourse.bass as bass  # noqa: F401
    import concourse.mybir as mybir
    import concourse.tile as tile
    from concourse import bacc
    from concourse.masks import make_identity

    f16 = mybir.dt.float16
    f32 = mybir.dt.float32
    AF = mybir.ActivationFunctionType

    nc = bacc.Bacc(
        "TRN2", target_bir_lowering=False, debug=False,
        num_devices=NC if collectives else 1,
    )

    # ---- I/O ----
    at_d = nc.dram_tensor("at_t", [KT_NODES, ND, P, 512], f16, kind="ExternalInput")
    x0_d = nc.dram_tensor("x0", [KT_NODES, P, T], f16, kind="ExternalInput")
    w_d = {}
    w_d["w1_0"] = nc.dram_tensor("w1_0", [H // P, P, T // P, P], f16, kind="ExternalInput")
    w_d["w2_0"] = nc.dram_tensor("w2_0", [H // P, P, H // P, P], f16, kind="ExternalInput")
    for l in (1, 2):
        w_d[f"w1_{l}"] = nc.dram_tensor(f"w1_{l}", [H // P, P, H // P, P], f16, kind="ExternalInput")
    w_d["w1_3"] = nc.dram_tensor("w1_3", [O // P, P, H // P, P], f16, kind="ExternalInput")
    for l in (1, 2):
        w_d[f"w2_{l}"] = nc.dram_tensor(f"w2_{l}", [H // P, P, H // P, P], f16, kind="ExternalInput")
    whm_d = nc.dram_tensor("whm", [P, O // P, P], f16, kind="ExternalInput")
    whv_d = nc.dram_tensor("whv", [P, O // P, P], f16, kind="ExternalInput")
    b_d = {}
    for l in range(3):
        b_d[f"b1_{l}"] = nc.dram_tensor(f"b1_{l}", [P, H // P], f32, kind="ExternalInput")
    b_d["b1_3"] = nc.dram_tensor("b1_3", [P, O // P], f32, kind="ExternalInput")
    for l in range(3):
        b_d[f"b2_{l}"] = nc.dram_tensor(f"b2_{l}", [P, H // P], f32, kind="ExternalInput")
    bhm_d = nc.dram_tensor("bhm", [P, 1], f32, kind="ExternalInput")
    bhv_d = nc.dram_tensor("bhv", [P, 1], f32, kind="ExternalInput")
    eps_d = nc.dram_tensor("epst", [P, NS], f32, kind="ExternalInput")

    z_d = nc.dram_tensor("zt", [P, NS], f32, kind="ExternalOutput")
    mean_d = nc.dram_tensor("meant", [P, NS], f32, kind="ExternalOutput")
    var_d = nc.dram_tensor("vart", [P, NS], f32, kind="ExternalOutput")

    HH = H // 2
    if dma_tp:
        # feature-major: xown [HH feats, NS nodes]; gathered [NC*HH, NS]
        xown = {(l, h): nc.dram_tensor(f"xown{l}_{h}", [HH, NS], f16)
                for l in (1, 2, 3) for h in (0, 1)}
        xg = {(l, h): nc.dram_tensor(f"xg{l}_{h}", [NC * HH, NS], f16, addr_space="Shared")
              for l in (1, 2, 3) for h in (0, 1)}
    else:
        xown = {(l, h): nc.dram_tensor(f"xown{l}_{h}", [NS, HH], f16)
                for l in (1, 2, 3) for h in (0, 1)}
        xg = {(l, h): nc.dram_tensor(f"xg{l}_{h}", [N, HH], f16, addr_space="Shared")
              for l in (1, 2, 3) for h in (0, 1)}

    rg = [list(range(NC))]

    with tile.TileContext(nc) as tc:
        with (
            tc.tile_pool(name="const", bufs=1) as const_p,
            tc.tile_pool(name="big", bufs=1) as big_p,
            tc.tile_pool(name="at", bufs=at_bufs) as at_p,
            tc.tile_pool(name="xslab", bufs=x_bufs) as x_p,
            tc.tile_pool(name="w", bufs=w_bufs) as w_p,
            tc.tile_pool(name="xo", bufs=2) as xo_p,
            tc.tile_pool(name="ps", bufs=ps_bufs, space="PSUM") as ps_p,
        ):
            ident = const_p.tile([P, P], f16, tag="ident")
            make_identity(nc, ident)

            bias_sb = {}
            for name, d in b_d.items():
                bias_sb[name] = const_p.tile(list(d.shape), f32, tag=f"b_{name}", name=f"b_{name}")
                nc.sync.dma_start(bias_sb[name][:], d[:])
            bhm_sb = const_p.tile([P, 1], f32, tag="bhm")
            nc.sync.dma_start(bhm_sb[:], bhm_d[:])
            bhv_sb = const_p.tile([P, 1], f32, tag="bhv")
            nc.sync.dma_start(bhv_sb[:], bhv_d[:])
            eps_sb = const_p.tile([P, NS], f32, tag="eps")
            nc.sync.dma_start(eps_sb[:], eps_d[:])
            whm_sb = const_p.tile([P, O // P, P], f16, tag="whm")
            nc.sync.dma_start(whm_sb[:], whm_d[:])
            whv_sb = const_p.tile([P, O // P, P], f16, tag="whv")
            nc.sync.dma_start(whv_sb[:], whv_d[:])

            def all_gather(l, h):
                if collectives:
                    nc.gpsimd.collective_compute(
                        "AllGather", mybir.AluOpType.bypass, replica_groups=rg,
                        ins=[xown[l, h][:].opt()], outs=[xg[l, h][:].opt()],
                    )
                else:
                    # sim-only stand-in: model the DMA traffic of the gather
                    for c in range(NC):
                        nc.sync.dma_start(xg[l, h][c * NS:(c + 1) * NS, :], xown[l, h][:])

            def agg(d_in, x_load_fn, uT, ns=None):
                """uT[:, mt, n*512:(n+1)*512] = sum_k x[k,m]^T @ AT[k,n]."""
                Mt = d_in // P
                for n in (range(ND) if ns is None else ns):
                    for g0 in range(0, Mt, agg_group):
                        gsz = min(agg_group, Mt - g0)
                        psums = [ps_p.tile([P, 512], f32, tag="mm", name=f"ps{_i}") for _i in range(gsz)]
                        for k in range(KT_NODES):
                            xs = x_p.tile([P, gsz * P], f16, tag="xslab")
                            x_load_fn(xs, k, g0 * P, gsz * P)
                            att = at_p.tile([P, 512], f16, tag="at")
                            nc.sync.dma_start(att[:], at_d[k, n])
                            for mi in range(gsz):
                                nc.tensor.matmul(
                                    psums[mi][:],
                                    lhsT=xs[:, mi * P:(mi + 1) * P],
                                    rhs=att[:],
                                    start=(k == 0),
                                    stop=(k == KT_NODES - 1),
                                )
                        for mi in range(gsz):
                            dst = uT[:, g0 + mi, n * 512:(n + 1) * 512]
                            if drain_split and mi % 2 == 1:
                                nc.scalar.copy(dst, psums[mi][:])
                            else:
                                nc.vector.tensor_copy(dst, psums[mi][:])

            def linear(w_dram, Kt, Mt, rhsT, outT, bias, relu, out_off=0, mts=None, ns=None):
                for mt in (range(Mt) if mts is None else mts):
                    ws = w_p.tile([P, Kt, P], f16, tag="w")
                    nc.sync.dma_start(ws[:], w_dram[mt])
                    for n in (range(ND) if ns is None else ns):
                        p = ps_p.tile([P, 512], f32, tag="mm")
                        for k in range(Kt):
                            nc.tensor.matmul(
                                p[:],
                                lhsT=ws[:, k, :],
                                rhs=rhsT[:, k, n * 512:(n + 1) * 512],
                                start=(k == 0),
                                stop=(k == Kt - 1),
                            )
                        nc.scalar.activation(
                            outT[:, out_off + mt, n * 512:(n + 1) * 512],
                            p[:],
                            AF.Relu if relu else AF.Identity,
                            bias=bias[:, mt:mt + 1],
                        )

            def transpose_store(xT, xown_dram, half):
                mt0 = half * (H // P // 2)
                nmt = H // P // 2
                if dma_tp:
                    # store feature-major directly; transposition happens on the
                    # post-AllGather XBAR load
                    for mt in range(nmt):
                        nc.sync.dma_start(
                            xown_dram[mt * P:(mt + 1) * P, :], xT[:, mt0 + mt, :]
                        )
                    return
                if no_transpose:
                    # sim-only: skip PE transposes, model DMA traffic directly
                    for j in range(NS // P):
                        for mt in range(nmt):
                            nc.sync.dma_start(
                                xown_dram[j * P:(j + 1) * P, mt * P:(mt + 1) * P],
                                xT[:, mt0 + mt, j * P:(j + 1) * P],
                            )
                    return
                for j in range(NS // P):
                    xo = xo_p.tile([P, nmt, P], f16, tag="xo")
                    for mt in range(nmt):
                        pt = ps_p.tile([P, P], f16, tag="mm")
                        nc.tensor.transpose(pt[:], xT[:, mt0 + mt, j * P:(j + 1) * P], ident[:])
                        if drain_split and mt % 2 == 1:
                            nc.scalar.copy(xo[:, mt, :], pt[:])
                        else:
                            nc.vector.tensor_copy(xo[:, mt, :], pt[:])
                    nc.sync.dma_start(xown_dram[j * P:(j + 1) * P, :], xo[:])

            uT0 = big_p.tile([P, T // P, NS], f16, tag="uT")
            hT = {}
            xT = {}

            # ---- layer 0 ----
            def x0_load(xs, k, c0, w):
                nc.sync.dma_start(xs[:], x0_d[k, :, c0:c0 + w])

            hT[0] = big_p.tile([P, H // P, NS], f16, tag="hT", name="hT0")
            xT[0] = big_p.tile([P, H // P, NS], f16, tag="xT", name="xT0")
            half0 = range(0, H // P // 2)
            half1 = range(H // P // 2, H // P)
            if not l0_split:
                with nc.named_scope("l0_agg"):
                    agg(T, x0_load, uT0)
                with nc.named_scope("l0_lin1"):
                    linear(w_d["w1_0"], T // P, H // P, uT0, hT[0], bias_sb["b1_0"], relu=True)
                for h, mts in ((0, half0), (1, half1)):
                    with nc.named_scope(f"l0_lin2_{h}"):
                        linear(w_d["w2_0"], H // P, H // P, hT[0], xT[0], bias_sb["b2_0"],
                               relu=True, mts=mts)
                    with nc.named_scope(f"l0_tp_{h}"):
                        transpose_store(xT[0], xown[1, h], h)
                    with nc.named_scope(f"ag1_{h}"):
                        all_gather(1, h)
            if l0_split:
              # layer 0's agg is AT-stream-bound (55us of MMs vs 90us of DMA), so
              # interleave its n-halves with MLP compute to cover the streaming
              with nc.named_scope("l0_agg0"):
                agg(T, x0_load, uT0, ns=[0])
              with nc.named_scope("l0_lin1_0"):
                  linear(w_d["w1_0"], T // P, H // P, uT0, hT[0], bias_sb["b1_0"],
                         relu=True, ns=[0])
              with nc.named_scope("l0_lin2_h0n0"):
                  linear(w_d["w2_0"], H // P, H // P, hT[0], xT[0], bias_sb["b2_0"],
                         relu=True, mts=half0, ns=[0])
              with nc.named_scope("l0_agg1"):
                  agg(T, x0_load, uT0, ns=[1])
              with nc.named_scope("l0_lin1_1"):
                  linear(w_d["w1_0"], T // P, H // P, uT0, hT[0], bias_sb["b1_0"],
                         relu=True, ns=[1])
              with nc.named_scope("l0_lin2_h0n1"):
                  linear(w_d["w2_0"], H // P, H // P, hT[0], xT[0], bias_sb["b2_0"],
                         relu=True, mts=half0, ns=[1])
              with nc.named_scope("l0_tp_0"):
                  transpose_store(xT[0], xown[1, 0], 0)
              with nc.named_scope("ag1_0"):
                  all_gather(1, 0)
              with nc.named_scope("l0_lin2_h1"):
                  linear(w_d["w2_0"], H // P, H // P, hT[0], xT[0], bias_sb["b2_0"],
                         relu=True, mts=half1)
              with nc.named_scope("l0_tp_1"):
                  transpose_store(xT[0], xown[1, 1], 1)
              with nc.named_scope("ag1_1"):
                  all_gather(1, 1)

            # ---- layers 1..3 ----
            for l in (1, 2, 3):
                uT = big_p.tile([P, H // P, NS], f16, tag="uT", name=f"uT{l}")
                with nc.named_scope(f"l{l}_agg"):
                    g0h, g1h = xg[l, 0], xg[l, 1]

                    def x_load(xs, k, c0, w, g0h=g0h, g1h=g1h):
                        gh = g0h if c0 < HH else g1h
                        c = c0 % HH
                        assert c + w <= HH
                        if dma_tp:
                            # xs[node, feat] <- XBAR-transposed [feat, node] block
                            r, j = k // (NS // P), k % (NS // P)
                            nc.sync.dma_start_transpose(
                                xs[:], gh[r * HH + c:r * HH + c + w, j * P:(j + 1) * P]
                            )
                        else:
                            nc.sync.dma_start(xs[:], gh[k * P:(k + 1) * P, c:c + w])

                    agg(H, x_load, uT)
                mt_out = (O if l == 3 else H) // P
                hT[l] = big_p.tile([P, mt_out, NS], f16, tag="hT", name=f"hTl{l}")
                with nc.named_scope(f"l{l}_lin1"):
                    linear(w_d[f"w1_{l}"], H // P, mt_out, uT, hT[l], bias_sb[f"b1_{l}"], relu=True)
                if l < 3:
                    xT[l] = big_p.tile([P, H // P, NS], f16, tag="xT", name=f"xTl{l}")
                    for h in (0, 1):
                        mts = range(h * (H // P // 2), (h + 1) * (H // P // 2))
                        with nc.named_scope(f"l{l}_lin2_{h}"):
                            linear(w_d[f"w2_{l}"], H // P, H // P, hT[l], xT[l],
                                   bias_sb[f"b2_{l}"], relu=True, mts=mts)
                        with nc.named_scope(f"l{l}_tp_{h}"):
                            transpose_store(xT[l], xown[l + 1, h], h)
                        with nc.named_scope(f"ag{l + 1}_{h}"):
                            all_gather(l + 1, h)

            # ---- fused heads ----
            mean_sb = const_p.tile([P, NS], f32, tag="mean_sb")
            var_sb = const_p.tile([P, NS], f32, tag="var_sb")
            z_sb = const_p.tile([P, NS], f32, tag="z_sb")
            with nc.named_scope("heads"):
                for W_sb, b_sb, o_sb in ((whm_sb, bhm_sb, mean_sb), (whv_sb, bhv_sb, var_sb)):
                    for n in range(ND):
                        p = ps_p.tile([P, 512], f32, tag="mm")
                        for k in range(O // P):
                            nc.tensor.matmul(
                                p[:],
                                lhsT=W_sb[:, k, :],
                                rhs=hT[3][:, k, n * 512:(n + 1) * 512],
                                start=(k == 0),
                                stop=(k == O // P - 1),
                            )
                        nc.scalar.activation(
                            o_sb[:, n * 512:(n + 1) * 512], p[:], AF.Identity,
                            bias=b_sb[:, 0:1],
                        )
                nc.vector.tensor_tensor(z_sb[:], var_sb[:], eps_sb[:], mybir.AluOpType.mult)
                nc.vector.tensor_tensor(z_sb[:], z_sb[:], mean_sb[:], mybir.AluOpType.add)
                nc.sync.dma_start(mean_d[:], mean_sb[:])
                nc.sync.dma_start(var_d[:], var_sb[:])
                nc.sync.dma_start(z_d[:], z_sb[:])

    nc.compile()
    return nc


def _tile_lhsT(w):
    """[K, M] fp16 -> [Mt, 128, Kt, 128]; slab [mt] is SBUF-ready [128p, Kt, 128m]."""
    K, M = w.shape
    Kt, Mt = K // P, M // P
    return np.ascontiguousarray(w.reshape(Kt, P, Mt, P).transpose(2, 1, 0, 3))


def _bias_t(b):
    """[M] fp32 -> [128, Mt] (partition = feature within tile)."""
    return np.ascontiguousarray(b.reshape(-1, P).T).astype(np.float32)


def prepare_inputs(inputs):
    """Host-side preprocessing: adjacency build + layout tiling. Returns in_maps."""
    f16 = np.float16
    eeg_nodes = np.asarray(inputs["eeg_nodes"], np.float32)
    eeg_idx = np.asarray(inputs["eeg_idx"])
    src = eeg_idx[0].astype(np.int64)
    dst = eeg_idx[1].astype(np.int64)

    counts = np.bincount(src * N + dst, minlength=N * N).reshape(N, N)
    AT = counts.astype(np.float32)
    AT[np.arange(N), np.arange(N)] += 1.0  # fold GIN's (1+eps)*x self-term, eps=0
    AT16 = AT.astype(f16)
    del AT, counts

    # Activations explode to ~1.3e5 by layer 3 (> fp16 max).  Since relu is
    # positively homogeneous, scale each of layers 0-2's output by S=1/16
    # (exact power of 2), folded into w2/b2; heads unscale via x S^-3.
    S = np.float32(1.0 / 16.0)
    c = [np.float32(1.0), S, S * S, S * S * S]  # cumulative scale of x_l input

    common = {}
    common["x0"] = np.ascontiguousarray(eeg_nodes.astype(f16).reshape(KT_NODES, P, T))
    for l in range(4):
        common[f"w1_{l}"] = _tile_lhsT(np.asarray(inputs[f"w1_{l}"], np.float32).astype(f16))
        common[f"b1_{l}"] = _bias_t(np.asarray(inputs[f"b1_{l}"], np.float32) * c[l])
    for l in range(3):
        common[f"w2_{l}"] = _tile_lhsT((np.asarray(inputs[f"w2_{l}"], np.float32) * S).astype(f16))
        common[f"b2_{l}"] = _bias_t(np.asarray(inputs[f"b2_{l}"], np.float32) * c[l + 1])

    # fused heads:  mean = h3 @ (w2_3 @ wm) + (b2_3 @ wm + bm); h3 arrives
    # scaled by c[3] so the fused weight is unscaled by 1/c[3].
    w2_3 = np.asarray(inputs["w2_3"], np.float32)
    b2_3 = np.asarray(inputs["b2_3"], np.float32)
    wm = np.asarray(inputs["wm"], np.float32)
    wv = np.asarray(inputs["wv"], np.float32)
    W2m = ((w2_3 @ wm) / c[3]).astype(f16)
    W2v = ((w2_3 @ wv) / c[3]).astype(f16)
    common["whm"] = _tile_lhsT(W2m)[0]
    common["whv"] = _tile_lhsT(W2v)[0]
    common["bhm"] = (b2_3 @ wm + np.asarray(inputs["bm"], np.float32)).reshape(P, 1).astype(np.float32)
    common["bhv"] = (b2_3 @ wv + np.asarray(inputs["bv"], np.float32)).reshape(P, 1).astype(np.float32)

    eps = np.asarray(inputs["eps"], np.float32)
    in_maps = []
    for c in range(NC):
        m = dict(common)
        blk = AT16[:, c * NS:(c + 1) * NS]
        m["at_t"] = np.ascontiguousarray(
            blk.reshape(KT_NODES, P, ND, 512).transpose(0, 2, 1, 3)
        )
        m["epst"] = np.ascontiguousarray(eps[c * NS:(c + 1) * NS, :].T)
        in_maps.append(m)
    return in_maps


def get_program():
    if "nc" not in _PROGRAM_CACHE:
        _PROGRAM_CACHE["nc"] = _build_program()
    return _PROGRAM_CACHE["nc"]


def assemble_outputs(results):
    z = np.empty((N, L), np.float32)
    mean = np.empty((N, L), np.float32)
    var = np.empty((N, L), np.float32)
    for c in range(NC):
        z[c * NS:(c + 1) * NS] = results[c]["zt"].T
        mean[c * NS:(c + 1) * NS] = results[c]["meant"].T
        var[c * NS:(c + 1) * NS] = results[c]["vart"].T
    return z, mean, var


def kernel(**inputs):
    from concourse.bass_utils import run_bass_kernel_spmd

    nc = get_program()
    in_maps = prepare_inputs(inputs)
    res = run_bass_kernel_spmd(nc, in_maps, core_ids=list(range(NC)))
    return assemble_outputs(res.results)



# revision 2
# speedup vs baseline: 1.0973x; 1.0973x over previous
"""Trainium2 Bass kernel for nn_DemLocGraphEncoder (4-layer GIN + variational heads).

v2 strategy (vs v1 baseline):
- Dense (I+A)^T aggregation matmul kept (N=8192 maps perfectly on the PE), but
  the adjacency streams as fp8e4 (multiplicities are small ints -> exact),
  halving its HBM traffic; mixed fp16(lhsT) x fp8(rhs) matmul runs at bf16 rate.
- lin2 runs NODE-major (hT chunks stationary, w2 moving, SBUF-resident):
  output lands node-major directly, eliminating ALL per-layer PE transposes.
  b2 is added via a K=1 ones-row matmul (outer-product broadcast).
- Layer 3 is pre-projected: y3 = w1_3^T relu(x2) is computed locally (O=1024
  wide), all-gathered, and aggregated at HALF the contraction cost; bias+relu
  fold into the agg drain.  Heads consume h3 directly (w2_3@{wm,wv} fused).
- AllGathers are split into 4 node-quarters per layer, fired as soon as each
  quarter's lin2 chunks drain; the next layer's agg k-order matches the
  quarter order so collectives overlap compute.
- DMA engine split: the two matmul operand streams (x slabs + adjacency
  tiles) issue from SP; weights, constants and stores issue from the
  Activation DGE so neither blocks the other at the SEQ.
- Nodes are row-sharded 1024/core; all src-node orderings (x0, adjacency rows)
  are host-permuted into the gathered (quarter-major, core-major) order.
"""

import sys

if "/opt/trn_rl_repo" not in sys.path:
    sys.path.insert(0, "/opt/trn_rl_repo")

import numpy as np

N, E, T, H, O, L = 8192, 262144, 256, 2048, 1024, 128
NC = 8
NS = N // NC          # 1024 nodes per core
P = 128
KT = N // P           # 64 k-tiles over (permuted) source nodes
ND = NS // 512        # 2 free-dim blocks over own nodes
NQ = 4                # all-gather quarters
QN = NS // NQ         # 256 nodes per quarter

FW = {1: H, 2: H, 3: O}   # gathered feature width entering layer l

_PROGRAM_CACHE = {}


def _build_program(collectives=True, opts=None):
    opts = dict(opts or {})
    import concourse.bass as bass  # noqa: F401
    import concourse.mybir as mybir
    import concourse.tile as tile
    from concourse import bacc
    from concourse.masks import make_identity

    f16 = mybir.dt.float16
    f32 = mybir.dt.float32
    f8 = mybir.dt.float8e4
    AF = mybir.ActivationFunctionType

    nc = bacc.Bacc(
        "TRN2", target_bir_lowering=False, debug=False,
        num_devices=NC if collectives else 1,
    )

    # ---- I/O ----
    at_d = nc.dram_tensor("at_t", [KT, P, ND, 512], f8, kind="ExternalInput")
    x0_d = nc.dram_tensor("x0", [KT, P, T], f16, kind="ExternalInput")
    w10_d = nc.dram_tensor("w1r_0", [P, T // P, H], f16, kind="ExternalInput")
    w1_d = {}
    for l in (1, 2):
        w1_d[l] = nc.dram_tensor(f"w1_{l}", [H // P, P, H // P, P], f16, kind="ExternalInput")
    w1_d[3] = nc.dram_tensor("w1_3", [O // P, P, H // P, P], f16, kind="ExternalInput")
    # w2 for l=0,1: rhs layout [featin_part, ktile, fout]
    w2r_d = {l: nc.dram_tensor(f"w2r_{l}", [P, H // P, H], f16, kind="ExternalInput")
             for l in (0, 1)}
    # w2 for l=2: lhsT slab layout (feature-major lin2)
    w2s_d = nc.dram_tensor("w2s_2", [H // P, P, H // P, P], f16, kind="ExternalInput")
    b1_d = {}
    for l in range(3):
        b1_d[l] = nc.dram_tensor(f"b1_{l}", [P, H // P], f32, kind="ExternalInput")
    b1_d[3] = nc.dram_tensor("b1_3", [P, O // P], f32, kind="ExternalInput")
    b2r_d = {l: nc.dram_tensor(f"b2r_{l}", [1, H], f16, kind="ExternalInput")
             for l in (0, 1)}
    b22_d = nc.dram_tensor("b2_2", [P, H // P], f32, kind="ExternalInput")
    whm_d = nc.dram_tensor("whm", [P, O // P, P], f16, kind="ExternalInput")
    whv_d = nc.dram_tensor("whv", [P, O // P, P], f16, kind="ExternalInput")
    bhm_d = nc.dram_tensor("bhm", [P, 1], f32, kind="ExternalInput")
    bhv_d = nc.dram_tensor("bhv", [P, 1], f32, kind="ExternalInput")
    eps_d = nc.dram_tensor("epst", [P, NS], f32, kind="ExternalInput")

    z_d = nc.dram_tensor("zt", [P, NS], f32, kind="ExternalOutput")
    mean_d = nc.dram_tensor("meant", [P, NS], f32, kind="ExternalOutput")
    var_d = nc.dram_tensor("vart", [P, NS], f32, kind="ExternalOutput")

    # per-quarter exchange buffers
    xo = {(l, q): nc.dram_tensor(f"xo{l}_{q}", [QN, FW[l]], f16)
          for l in (1, 2, 3) for q in range(NQ)}
    xg = {(l, q): nc.dram_tensor(f"xg{l}_{q}", [NC * QN, FW[l]], f16, addr_space="Shared")
          for l in (1, 2, 3) for q in range(NQ)}

    rg = [list(range(NC))]

    with tile.TileContext(nc) as tc:
        with (
            tc.tile_pool(name="const", bufs=1) as const_p,
            tc.tile_pool(name="w2f", bufs=1) as w2_p,
            tc.tile_pool(name="big", bufs=1) as big_p,
            tc.tile_pool(name="xslab", bufs=5) as x_p,
            tc.tile_pool(name="at", bufs=6) as at_p,
            tc.tile_pool(name="w1", bufs=3) as w1_p,
            tc.tile_pool(name="xo", bufs=2) as xo_p,
            tc.tile_pool(name="yo", bufs=2) as yo_p,
            tc.tile_pool(name="ps", bufs=8, space="PSUM") as ps_p,
        ):
            # --- constants: all on the ACT DGE so SP starts streaming x0/at ---
            ident = const_p.tile([P, P], f16, tag="ident")
            make_identity(nc, ident)
            ones_sb = const_p.tile([1, P], f16, tag="ones")
            nc.vector.memset(ones_sb[:], 1.0)

            w10_sb = const_p.tile([P, T // P, H], f16, tag="w10")
            nc.scalar.dma_start(w10_sb[:], w10_d[:])
            b1_sb = {}
            for l, d in b1_d.items():
                b1_sb[l] = const_p.tile(list(d.shape), f32, tag=f"b1_{l}", name=f"b1_{l}")
                nc.scalar.dma_start(b1_sb[l][:], d[:])
            b2r_sb = {}
            for l, d in b2r_d.items():
                b2r_sb[l] = const_p.tile([1, H], f16, tag=f"b2r_{l}", name=f"b2r_{l}")
                nc.scalar.dma_start(b2r_sb[l][:], d[:])
            b22_sb = const_p.tile([P, H // P], f32, tag="b22")
            nc.scalar.dma_start(b22_sb[:], b22_d[:])
            bhm_sb = const_p.tile([P, 1], f32, tag="bhm")
            nc.scalar.dma_start(bhm_sb[:], bhm_d[:])
            bhv_sb = const_p.tile([P, 1], f32, tag="bhv")
            nc.scalar.dma_start(bhv_sb[:], bhv_d[:])
            eps_sb = const_p.tile([P, NS], f32, tag="eps")
            nc.scalar.dma_start(eps_sb[:], eps_d[:])
            whm_sb = const_p.tile([P, O // P, P], f16, tag="whm")
            nc.scalar.dma_start(whm_sb[:], whm_d[:])
            whv_sb = const_p.tile([P, O // P, P], f16, tag="whv")
            nc.scalar.dma_start(whv_sb[:], whv_d[:])

            w2f = {}
            for l in (0, 1):
                w2f[l] = w2_p.tile([P, H // P, H], f16, tag="w2f", name=f"w2f_{l}")

            def load_w2f(l):
                for kk in range(0, H // P, 2):
                    nc.scalar.dma_start(w2f[l][:, kk:kk + 2, :], w2r_d[l][:, kk:kk + 2, :])

            load_w2f(0)  # on ACT queue: off the x0/at critical path

            def all_gather(l, q):
                if collectives:
                    nc.gpsimd.collective_compute(
                        "AllGather", mybir.AluOpType.bypass, replica_groups=rg,
                        ins=[xo[l, q][:].opt()], outs=[xg[l, q][:].opt()],
                    )
                else:
                    for c in range(NC):
                        nc.sync.dma_start(xg[l, q][c * QN:(c + 1) * QN, :], xo[l, q][:])

            def agg(d_in, x_load_fn, n, drain_fn):
                """psum[mi] = sum_k xs[k, g0+mi]^T @ AT[k, n];  drain via drain_fn."""
                Mt = d_in // P
                for g0 in range(0, Mt, 8):
                    gsz = min(8, Mt - g0)
                    psums = [ps_p.tile([P, 512], f32, tag="mm", name=f"ps{i}")
                             for i in range(gsz)]
                    for k in range(KT):
                        xs = x_p.tile([P, gsz * P], f16, tag="xslab")
                        x_load_fn(xs, k, g0 * P, gsz * P)
                        att = at_p.tile([P, 512], f8, tag="at")
                        nc.sync.dma_start(att[:], at_d[k, :, n, :])
                        for mi in range(gsz):
                            nc.tensor.matmul(
                                psums[mi][:],
                                lhsT=xs[:, mi * P:(mi + 1) * P],
                                rhs=att[:],
                                start=(k == 0),
                                stop=(k == KT - 1),
                            )
                    drain_fn(psums, g0)

            def drain_uT(uT, off=0):
                def fn(psums, g0):
                    for mi, p in enumerate(psums):
                        dst = uT[:, off + g0 + mi, :]
                        if mi % 2 == 1:
                            nc.scalar.copy(dst, p[:])
                        else:
                            nc.vector.tensor_copy(dst, p[:])
                return fn

            def linear_fmajor(w_dram, Kt, Mt, rhsT, outT, func, bias_sb=None,
                              rhs_off=0):
                """outT[:, mt, :] = func(sum_k w[mt,k]^T @ rhsT[:, k, :] + b[mt])."""
                for mt in range(Mt):
                    ws = w1_p.tile([P, Kt, P], f16, tag="w1")
                    nc.scalar.dma_start(ws[:], w_dram[mt])
                    p = ps_p.tile([P, 512], f32, tag="mm")
                    for k in range(Kt):
                        nc.tensor.matmul(
                            p[:], lhsT=ws[:, k, :], rhs=rhsT[:, rhs_off + k, :],
                            start=(k == 0), stop=(k == Kt - 1),
                        )
                    nc.scalar.activation(
                        outT[:, mt, :], p[:], func,
                        bias=bias_sb[:, mt:mt + 1] if bias_sb is not None else 0.0,
                    )

            def lin2_nodemajor(l, hT, n, xol):
                """x_{l+1}[chunk, :] = relu(hT_chunk^T @ w2 + b2); node-major out."""
                for ch in range(4):
                    pb = [ps_p.tile([P, 512], f32, tag="mm", name=f"pb{fg}")
                          for fg in range(4)]
                    for fg in range(4):
                        nc.tensor.matmul(
                            pb[fg][:], lhsT=ones_sb[:],
                            rhs=b2r_sb[l][:, fg * 512:(fg + 1) * 512],
                            start=True, stop=False,
                        )
                    for k in range(H // P):
                        for fg in range(4):
                            nc.tensor.matmul(
                                pb[fg][:],
                                lhsT=hT[:, k, ch * P:(ch + 1) * P],
                                rhs=w2f[l][:, k, fg * 512:(fg + 1) * 512],
                                start=False, stop=(k == H // P - 1),
                            )
                    xo_t = xo_p.tile([P, H], f16, tag="xo")
                    for fg in range(4):
                        if fg % 2 == 1:
                            nc.scalar.activation(
                                xo_t[:, fg * 512:(fg + 1) * 512], pb[fg][:], AF.Relu)
                        else:
                            nc.vector.tensor_scalar_max(
                                xo_t[:, fg * 512:(fg + 1) * 512], pb[fg][:], 0.0)
                    gci = n * 4 + ch
                    q, r0 = gci // 2, (gci % 2) * P
                    nc.scalar.dma_start(xo[xol, q][r0:r0 + P, :], xo_t[:])
                    if gci % 2 == 1:
                        with nc.named_scope(f"ag{xol}_{q}"):
                            all_gather(xol, q)

            def xg_load(l):
                def fn(xs, k, c0, w):
                    src = xg[l, k // 16]
                    r0 = (k % 16) * P
                    nc.sync.dma_start(xs[:], src[r0:r0 + P, c0:c0 + w])
                return fn

            uT = big_p.tile([P, H // P, 512], f16, tag="uT")
            hT = big_p.tile([P, H // P, 512], f16, tag="hT")
            x2T = big_p.tile([P, H // P, 512], f16, tag="x2T")
            y3T = big_p.tile([P, O // P, 512], f16, tag="y3T")
            hT3 = big_p.tile([P, O // P, 512], f16, tag="hT3")
            mean_sb = const_p.tile([P, NS], f32, tag="mean_sb")
            var_sb = const_p.tile([P, NS], f32, tag="var_sb")
            z_sb = const_p.tile([P, NS], f32, tag="z_sb")

            # ---- layer 0: agg n-inner (x0 + at streamed once), then per-n MLP ----
            with nc.named_scope("l0_agg"):
                psums = [ps_p.tile([P, 512], f32, tag="mm", name=f"ps0{i}")
                         for i in range(2 * ND)]
                for k in range(KT):
                    xs = x_p.tile([P, T], f16, tag="xslab")
                    nc.sync.dma_start(xs[:], x0_d[k])
                    att2 = at_p.tile([P, ND, 512], f8, tag="at", name="at0")
                    nc.sync.dma_start(att2[:], at_d[k])
                    for n in range(ND):
                        for mi in range(2):
                            nc.tensor.matmul(
                                psums[n * 2 + mi][:],
                                lhsT=xs[:, mi * P:(mi + 1) * P],
                                rhs=att2[:, n, :],
                                start=(k == 0), stop=(k == KT - 1),
                            )
                for n in range(ND):
                    drain_uT(uT, off=2 * n)(psums[2 * n:2 * n + 2], 0)
            for n in range(ND):
                with nc.named_scope(f"l0_lin1_{n}"):
                    # resident w1_0: lhsT slices from [P, 2, H]
                    for mt in range(H // P):
                        p = ps_p.tile([P, 512], f32, tag="mm")
                        for k in range(T // P):
                            nc.tensor.matmul(
                                p[:], lhsT=w10_sb[:, k, mt * P:(mt + 1) * P],
                                rhs=uT[:, 2 * n + k, :],
                                start=(k == 0), stop=(k == T // P - 1),
                            )
                        nc.scalar.activation(hT[:, mt, :], p[:], AF.Relu,
                                             bias=b1_sb[0][:, mt:mt + 1])
                with nc.named_scope(f"l0_lin2_{n}"):
                    lin2_nodemajor(0, hT, n, 1)

            # ---- layers 1..2 ----
            for l in (1, 2):
                if l == 1:
                    load_w2f(1)
                for n in range(ND):
                    with nc.named_scope(f"l{l}_agg{n}"):
                        agg(H, xg_load(l), n, drain_uT(uT))
                    with nc.named_scope(f"l{l}_lin1_{n}"):
                        linear_fmajor(w1_d[l], H // P, H // P, uT, hT,
                                      AF.Relu, b1_sb[l])
                    if l == 1:
                        with nc.named_scope(f"l1_lin2_{n}"):
                            lin2_nodemajor(1, hT, n, 2)
                    else:
                        with nc.named_scope(f"l2_lin2_{n}"):
                            linear_fmajor(w2s_d, H // P, H // P, hT, x2T,
                                          AF.Relu, b22_sb)
                        with nc.named_scope(f"y3_{n}"):
                            linear_fmajor(w1_d[3], H // P, O // P, x2T, y3T,
                                          AF.Identity)
                        with nc.named_scope(f"y3tp_{n}"):
                            for j in range(4):
                                yo_t = yo_p.tile([P, O // P, P], f16, tag="yo")
                                for mt in range(O // P):
                                    pt = ps_p.tile([P, P], f16, tag="mm")
                                    nc.tensor.transpose(
                                        pt[:], y3T[:, mt, j * P:(j + 1) * P], ident[:])
                                    if mt % 2 == 1:
                                        nc.scalar.copy(yo_t[:, mt, :], pt[:])
                                    else:
                                        nc.vector.tensor_copy(yo_t[:, mt, :], pt[:])
                                gci = n * 4 + j
                                q, r0 = gci // 2, (gci % 2) * P
                                nc.scalar.dma_start(xo[3, q][r0:r0 + P, :], yo_t[:])
                                if gci % 2 == 1:
                                    with nc.named_scope(f"ag3_{q}"):
                                        all_gather(3, q)

            # ---- layer 3: agg over y3 (O-wide), bias+relu at drain, heads ----
            for n in range(ND):
                def drain_h3(psums, g0):
                    for mi, p in enumerate(psums):
                        nc.scalar.activation(
                            hT3[:, g0 + mi, :], p[:], AF.Relu,
                            bias=b1_sb[3][:, g0 + mi:g0 + mi + 1],
                        )
                with nc.named_scope(f"l3_agg{n}"):
                    agg(O, xg_load(3), n, drain_h3)
                with nc.named_scope(f"heads_{n}"):
                    for W_sb, b_sb, o_sb in ((whm_sb, bhm_sb, mean_sb),
                                             (whv_sb, bhv_sb, var_sb)):
                        p = ps_p.tile([P, 512], f32, tag="mm")
                        for k in range(O // P):
                            nc.tensor.matmul(
                                p[:], lhsT=W_sb[:, k, :], rhs=hT3[:, k, :],
                                start=(k == 0), stop=(k == O // P - 1),
                            )
                        nc.scalar.activation(
                            o_sb[:, n * 512:(n + 1) * 512], p[:], AF.Identity,
                            bias=b_sb[:, 0:1],
                        )

            with nc.named_scope("zout"):
                nc.vector.tensor_tensor(z_sb[:], var_sb[:], eps_sb[:], mybir.AluOpType.mult)
                nc.vector.tensor_tensor(z_sb[:], z_sb[:], mean_sb[:], mybir.AluOpType.add)
                nc.sync.dma_start(mean_d[:], mean_sb[:])
                nc.sync.dma_start(var_d[:], var_sb[:])
                nc.sync.dma_start(z_d[:], z_sb[:])

    nc.compile()
    return nc


def _tile_lhsT(w):
    """[K, M] fp16 -> [Mt, 128, Kt, 128]; slab [mt] is SBUF-ready [128p, Kt, 128m]."""
    K, M = w.shape
    Kt, Mt = K // P, M // P
    return np.ascontiguousarray(w.reshape(Kt, P, Mt, P).transpose(2, 1, 0, 3))


def _rhs_tiles(w):
    """[K, M] fp16 -> [128, Kt, M]: rhs tile layout, contraction on partitions."""
    K, M = w.shape
    Kt = K // P
    return np.ascontiguousarray(w.reshape(Kt, P, M).transpose(1, 0, 2))


def _bias_t(b):
    """[M] fp32 -> [128, Mt] (partition = feature within tile)."""
    return np.ascontiguousarray(b.reshape(-1, P).T).astype(np.float32)


def _perm():
    return np.concatenate([c * NS + q * QN + np.arange(QN)
                           for q in range(NQ) for c in range(NC)])


def prepare_inputs(inputs):
    """Host-side preprocessing: adjacency build + layout tiling. Returns in_maps."""
    f16 = np.float16
    import ml_dtypes
    f8 = ml_dtypes.float8_e4m3

    eeg_nodes = np.asarray(inputs["eeg_nodes"], np.float32)
    eeg_idx = np.asarray(inputs["eeg_idx"])
    src = eeg_idx[0].astype(np.int64)
    dst = eeg_idx[1].astype(np.int64)

    counts = np.bincount(src * N + dst, minlength=N * N).reshape(N, N)
    AT = counts.astype(np.float32)
    AT[np.arange(N), np.arange(N)] += 1.0  # fold GIN's (1+eps)*x self-term, eps=0
    perm = _perm()
    ATp = np.ascontiguousarray(AT[perm, :]).astype(f8)
    del AT, counts

    # Activations explode to ~1.3e5 by layer 3 (> fp16 max).  Since relu is
    # positively homogeneous, scale each of layers 0-2's output by S=1/16
    # (exact power of 2), folded into w2/b2; heads unscale via x S^-3.
    S = np.float32(1.0 / 16.0)
    c = [np.float32(1.0), S, S * S, S * S * S]  # cumulative scale of x_l input

    common = {}
    common["x0"] = np.ascontiguousarray(
        eeg_nodes[perm].astype(f16).reshape(KT, P, T))
    common["w1r_0"] = _rhs_tiles(np.asarray(inputs["w1_0"], np.float32).astype(f16))
    for l in (1, 2, 3):
        common[f"w1_{l}"] = _tile_lhsT(np.asarray(inputs[f"w1_{l}"], np.float32).astype(f16))
    for l in range(4):
        common[f"b1_{l}"] = _bias_t(np.asarray(inputs[f"b1_{l}"], np.float32) * c[l])
    for l in (0, 1):
        w2 = (np.asarray(inputs[f"w2_{l}"], np.float32) * S).astype(f16)
        common[f"w2r_{l}"] = _rhs_tiles(w2)
        common[f"b2r_{l}"] = (np.asarray(inputs[f"b2_{l}"], np.float32)
                              * c[l + 1]).astype(f16).reshape(1, H)
    common["w2s_2"] = _tile_lhsT((np.asarray(inputs["w2_2"], np.float32) * S).astype(f16))
    common["b2_2"] = _bias_t(np.asarray(inputs["b2_2"], np.float32) * c[3])

    # fused heads:  mean = h3 @ (w2_3 @ wm) + (b2_3 @ wm + bm); h3 arrives
    # scaled by c[3] so the fused weight is unscaled by 1/c[3].
    w2_3 = np.asarray(inputs["w2_3"], np.float32)
    b2_3 = np.asarray(inputs["b2_3"], np.float32)
    wm = np.asarray(inputs["wm"], np.float32)
    wv = np.asarray(inputs["wv"], np.float32)
    common["whm"] = _tile_lhsT(((w2_3 @ wm) / c[3]).astype(f16))[0]
    common["whv"] = _tile_lhsT(((w2_3 @ wv) / c[3]).astype(f16))[0]
    common["bhm"] = (b2_3 @ wm + np.asarray(inputs["bm"], np.float32)).reshape(P, 1).astype(np.float32)
    common["bhv"] = (b2_3 @ wv + np.asarray(inputs["bv"], np.float32)).reshape(P, 1).astype(np.float32)

    eps = np.asarray(inputs["eps"], np.float32)
    in_maps = []
    for cc in range(NC):
        m = dict(common)
        blk = ATp[:, cc * NS:(cc + 1) * NS]
        m["at_t"] = np.ascontiguousarray(blk.reshape(KT, P, ND, 512))
        m["epst"] = np.ascontiguousarray(eps[cc * NS:(cc + 1) * NS, :].T)
        in_maps.append(m)
    return in_maps


def get_program():
    if "nc" not in _PROGRAM_CACHE:
        _PROGRAM_CACHE["nc"] = _build_program()
    return _PROGRAM_CACHE["nc"]


def assemble_outputs(results):
    z = np.empty((N, L), np.float32)
    mean = np.empty((N, L), np.float32)
    var = np.empty((N, L), np.float32)
    for c in range(NC):
        z[c * NS:(c + 1) * NS] = results[c]["zt"].T
        mean[c * NS:(c + 1) * NS] = results[c]["meant"].T
        var[c * NS:(c + 1) * NS] = results[c]["vart"].T
    return z, mean, var


def kernel(**inputs):
    from concourse.bass_utils import run_bass_kernel_spmd

    nc = get_program()
    in_maps = prepare_inputs(inputs)
    res = run_bass_kernel_spmd(nc, in_maps, core_ids=list(range(NC)))
    return assemble_outputs(res.results)
